# revision 41
# baseline (speedup 1.0000x reference)
"""Causal multi-head attention (B=32,T=512,C=1024,H=16,D=64) on 8 TRN2 cores.

Strategy: pure data-parallel over the batch axis (4 batches per core, no
collectives). Per core, per batch:
  - x^T [C,T] arrives pre-transposed from the host (layout prep only).
  - Q^T [HD,T] / K^T / V computed with bf16 matmuls (fp32 PSUM). K^T keeps
    its natural paired layout: head 2m on partitions 0:64, head 2m+1 on
    64:128 of kTp[:, m, :] -- a single [128,T] PSUM copy per chunk.
  - scores^T for a PAIR of heads computed with row-tiled matmuls: head 2m
    uses PE rows 0:63 (tile_position (0,0), K=64) and head 2m+1 rows 64:127
    (tile_position (64,0)) CONCURRENTLY -- 2x scores throughput vs the
    zero-padded K=128 form, and no kT zero-fill memsets.
  - scores PSUM layout: per-parity [128,1024] tile (j0@[0,512) bank A,
    j1@[512,896)+j3@[896,1024) bank B) plus a shared [128,2,256] tile for
    the j2 chunk (one bank, even/odd halves) -- exp is 2 ACTs per head into
    the same aT offsets as a packed [128,1280] tile would use.
  - softmax without max-subtraction (scores bounded); masked entries zeroed
    by multiplying exp'd diagonal blocks with 0/1 masks on GpSimd.
  - attn@V with a ones-augmented V column producing softmax row-sums in the
    same matmul; all four t-chunks accumulate into ONE dedicated PSUM bank
    (psAV, its own bank so projection-chain PSUM never WAR-waits on the
    normalize chain -- this was the baseline's 1.3us/slot PE stall).
  - head-concat transpose via batched DMA-transpose; final projection with
    bias folded into the PSUM evacuation; bf16 output (host casts to f32).

PSUM budget (8 banks): psS_even 2 + psS_odd 2 + psJ2 1 + psAV 1 + psq 2.

Scheduling:
  - initial DMAs spread across 4 queues (sync/scalar/vector/gpsimd) so the
    first Q matmul is gated by ~wq+xT arrival only.
  - warm-up junk matmuls at t=0 keep the PE busy during the initial DMA so
    the HAM clock-gate reaches 8/8 before real work lands.
  - software pipeline, two levels:
      * within attention: AV(h-3) issues while scores(pair) runs; scalar
        engine kept exp-only; DVE emission order per slot puts the AV
        normalize (recip + 4 scalar-muls) BEFORE filler-chain evacuations;
      * across batches: batch b+1's projection chains interleave batch b's
        attention head slots and output projection.
"""

import sys

if "/opt/trn_rl_repo" not in sys.path:
    sys.path.insert(0, "/opt/trn_rl_repo")

import numpy as np
import ml_dtypes

B, T, C = 32, 512, 1024
H, D = 16, 64
HD = H * D
NCORES = 8
B_LOC = B // NCORES

_CACHE = {}


def build_nc(b_loc=B_LOC):
    import concourse.mybir as mybir
    from concourse import bacc
    from concourse.bass import ds, ts
    from concourse.tile import TileContext

    f32 = mybir.dt.float32
    bf16 = mybir.dt.bfloat16
    fp8 = mybir.dt.float8e4
    DR = mybir.MatmulPerfMode.DoubleRow
    AF = mybir.ActivationFunctionType

    KO = C // 128  # 8 contraction chunks
    KK = C // 256  # 4 DoubleRow contraction chunks (256 rows each)
    MO = HD // 128  # 8 output-row chunks
    TCH = T // 128  # 4 t-chunks
    NPAIR = H // 2
    # wq/wk host-scaled by 32 => scores inflated 1024x; fold into exp
    SCALE = 1.0 / float(np.sqrt(C)) / 1024.0
    N_WARM = 26
    LAG = 3  # attention software-pipeline depth (AV trails scores by LAG)
    ROWTILE = False  # row-tiled K=64 pair scores vs zero-padded K=128

    # scores^T causal layout: s-chunk j covers t in [128j, T), width T-128j.
    # aT (SBUF) offsets: j0@0, j1@512, j3@896, j2@1024 (packed [128,1280]).
    # PSUM: j0/j1/j3 in a [128,1024] per-parity tile (2 banks, no matmul
    # crosses a bank), j2 (width 256) in a shared [128,2,256] one-bank tile.
    widths = [T - 128 * j for j in range(TCH)]
    off = [0, 512, 1024, 896]  # aT column offset per j
    sOFF = {0: 0, 1: 512, 3: 896}  # psS column offset per j (j2 -> psJ2)
    PACK = 1280

    nc = bacc.Bacc("TRN2", target_bir_lowering=False)
    xT = nc.dram_tensor("xT", [b_loc, C, T], bf16, kind="ExternalInput")
    # fp8 copies for the Q/K projections (DoubleRow, 2x PE throughput);
    # wq/wk arrive host-scaled by 32 so their N(0,1/1024) entries use the
    # e4m3 normal range -- the 1024x score inflation is folded into the
    # exp's scale. Softmax damping keeps the fp8 error ~1e-2 end to end.
    xT8 = nc.dram_tensor("xT8", [b_loc, C, T], fp8, kind="ExternalInput")
    wq = nc.dram_tensor("wq", [C, HD], fp8, kind="ExternalInput")
    wk = nc.dram_tensor("wk", [C, HD], fp8, kind="ExternalInput")
    wv = nc.dram_tensor("wv", [C, HD], bf16, kind="ExternalInput")
    wp = nc.dram_tensor("wp", [C, C], bf16, kind="ExternalInput")
    bp = nc.dram_tensor("bp", [1, C], bf16, kind="ExternalInput")
    mask = nc.dram_tensor("mask", [128, 128], bf16, kind="ExternalInput")
    # [tri | ones(256) | tri]: masks the j1 and j3 diagonal blocks (packed at
    # columns 512:640 and 896:1024 of aT) in one elementwise multiply.
    mask512 = nc.dram_tensor("mask512", [128, 512], bf16, kind="ExternalInput")
    out = nc.dram_tensor("out", [b_loc, T, C], bf16, kind="ExternalOutput")

    with TileContext(nc) as tc:
        with (
            tc.tile_pool(name="weights", bufs=1) as wpool,
            tc.tile_pool(name="acts", bufs=2) as xpool,
            tc.tile_pool(name="attn", bufs=1) as apool,
            tc.tile_pool(name="small", bufs=8) as spool,
            tc.tile_pool(name="ons", bufs=2) as onpool,
            tc.tile_pool(name="outs", bufs=2) as opool,
            tc.tile_pool(name="psScores", bufs=1, space="PSUM") as psS,
            tc.tile_pool(name="psChain", bufs=2, space="PSUM") as psQ,
        ):
            # ---- persistent scores/AV PSUM tiles (bank-exact) ----
            # Row-tiled pairs: the two concurrent PE row-tiles must NEVER
            # touch the same PSUM bank (HW restriction), so even-head scores
            # go to banks 0,1,4 and odd-head scores to banks 2,3,5. The AV
            # accumulator (written by non-tiled matmuls, which never overlap
            # tiled ones thanks to the mode-switch drain) fills the back
            # halves of banks 4 and 5.
            psS_par = [
                psS.tile([128, 1024], f32, name="psS_e"),  # banks 0-1
                psS.tile([128, 1024], f32, name="psS_o"),  # banks 2-3
            ]
            # The two concurrent row tiles must have fully disjoint PSUM
            # bank sets (sharing one bank hangs the PE, even when the two
            # tiles' matmuls are emitted far apart). Non-tiled matmuls (AV)
            # may share a bank with a row tile (mode switches drain), so the
            # AV accumulator chunks fill the back halves of banks 4 and 5.
            psB4 = psS.tile([128, 512], f32, name="psB4")  # bank 4 (even j2)
            psB5 = psS.tile([128, 512], f32, name="psB5")  # bank 5 (odd j2)
            psJ2 = [psB4[:, 0:256], psB5[:, 0:256]]
            # AV chunks (68-wide, 16B-aligned): i=0..2 in bank 5, i=3 bank 4
            def psAV_chunk(i, w=65):
                if i < 3:
                    return psB5[:, ds(256 + 68 * i, w)]
                return psB4[:, ds(256, w)]

            # ---- persistent weight tiles ----
            # wq/wk in fp8 DoubleRow layout: [p, kk, i, n] with contraction
            # row c = kk*256 + i*128 + p
            wq_sb = wpool.tile([128, KK, 2, HD], fp8, name="wq_sb")
            wk_sb = wpool.tile([128, KK, 2, HD], fp8, name="wk_sb")
            wv_sb = wpool.tile([128, KO, HD], bf16, name="wv_sb")
            wp_sb = wpool.tile([128, KO, C], bf16, name="wp_sb")
            bp1_sb = wpool.tile([1, C], bf16, name="bp1_sb")
            mask_sb = wpool.tile([128, 128], bf16, name="mask_sb")
            m512_sb = wpool.tile([128, 512], bf16, name="m512_sb")
            # warm-up junk operand: zeros, written by the (fast) vector memset
            warm_sb = wpool.tile([128, 512], bf16, name="warm_sb")
            nc.vector.memset(warm_sb, 0.0)
            ones1_sb = wpool.tile([1, 128], bf16, name="ones1_sb")
            nc.gpsimd.memset(ones1_sb, 1.0)
            # K^T paired layout (head 2m -> partitions 0:64, 2m+1 -> 64:128);
            # two persistent slots for cross-batch overlap. No zero padding.
            if ROWTILE:
                kTp_tiles = [
                    wpool.tile([128, NPAIR, T], bf16, name=f"kTp_{s}")
                    for s in range(2)
                ]
            else:
                # zero-padded per-head K^T (K=128 matmul fallback)
                kT2_tiles = []
                for slot in range(2):
                    t_ = wpool.tile([128, H, T], bf16, name=f"kT2_{slot}")
                    nc.gpsimd.memset(t_, 0.0)
                    kT2_tiles.append(t_)

            # ---- initial DMA issues: weights on the sync queue, x on the
            # scalar queue -- two DMA channels in parallel. Steady-state xT
            # prefetches also ride scalar, keeping the sync queue free for
            # the concat-transposes that gate the output projection.
            nc.sync.dma_start(out=bp1_sb, in_=bp[:])
            nc.sync.dma_start(out=mask_sb, in_=mask[:])
            nc.sync.dma_start(out=m512_sb, in_=mask512[:])
            xT_tiles = {}
            xT8_tiles = {}
            nc.sync.dma_start(
                out=wq_sb[:, 0:2, :, :],
                in_=wq[0:512, :].rearrange("(kk i p) n -> p kk i n", p=128, i=2),
            )
            nc.sync.dma_start(
                out=wq_sb[:, 2:4, :, :],
                in_=wq[512:1024, :].rearrange("(kk i p) n -> p kk i n", p=128, i=2),
            )
            nc.sync.dma_start(
                out=wk_sb, in_=wk[:].rearrange("(kk i p) n -> p kk i n", p=128, i=2)
            )
            nc.sync.dma_start(
                out=wv_sb[:, 0:4, :],
                in_=wv[0:512, :].rearrange("(ko p) n -> p ko n", p=128),
            )
            nc.sync.dma_start(
                out=wv_sb[:, 4:8, :],
                in_=wv[512:1024, :].rearrange("(ko p) n -> p ko n", p=128),
            )
            nc.sync.dma_start(
                out=wp_sb, in_=wp[:].rearrange("(ko p) n -> p ko n", p=128)
            )

            def issue_xT(b):
                if b >= b_loc or b in xT_tiles:
                    return
                xT_tiles[b] = xpool.tile([128, KO, T], bf16, name="xT_sb", tag="xT")
                xT8_tiles[b] = xpool.tile(
                    [128, KK, 2, T], fp8, name="xT8_sb", tag="xT8"
                )
                nc.scalar.dma_start(
                    out=xT8_tiles[b],
                    in_=xT8[b].rearrange("(kk i p) t -> p kk i t", p=128, i=2),
                )
                nc.scalar.dma_start(
                    out=xT_tiles[b],
                    in_=xT[b].rearrange("(ko p) t -> p ko t", p=128),
                )

            issue_xT(0)
            issue_xT(1)

            # ---- PE warm-up: junk matmuls so the HAM clock-gate opens while
            # the initial DMA streams in. Output is never read.
            for w in range(N_WARM):
                psw = psQ.tile([128, 512], f32, name="psw", tag="psq")
                nc.tensor.matmul(
                    psw, warm_sb[:, 0:128], warm_sb, start=True, stop=True
                )

            # ---- bias broadcast to all 128 partitions, f32, built once ----
            bias_bc = wpool.tile([128, C], f32, name="bias_bc")
            for half in range(2):
                psb = psQ.tile([128, 512], f32, name="psb", tag="psq")
                nc.tensor.matmul(
                    psb, ones1_sb, bp1_sb[:, ts(half, 512)], start=True, stop=True
                )
                nc.vector.tensor_copy(out=bias_bc[:, ts(half, 512)], in_=psb)

            # ---------- pipeline-stage chain generators ----------
            qT_tiles = {}
            v_tiles = {}

            def q_chain(b, m):
                """Q^T projection chunk m -> qT[b][:, m, :] (DVE evac).
                fp8 DoubleRow: 4 matmuls of K=256 each."""
                if b not in qT_tiles:
                    qT_tiles[b] = xpool.tile(
                        [128, MO, T], bf16, name="qT_sb", tag="qT"
                    )
                ps = psQ.tile([128, T], f32, name="ps_q", tag="psq")
                for kk in range(KK):
                    nc.tensor.matmul(
                        ps,
                        wq_sb[:, kk, :, ts(m, 128)],
                        xT8_tiles[b][:, kk, :, :],
                        start=(kk == 0),
                        stop=(kk == KK - 1),
                        perf_mode=DR,
                    )
                nc.vector.tensor_copy(out=qT_tiles[b][:, m, :], in_=ps)

            def k_chain(b, m):
                """K^T chunk m -> kTp (paired layout IS the PSUM layout)."""
                ps = psQ.tile([128, T], f32, name="ps_k", tag="psq")
                for kk in range(KK):
                    nc.tensor.matmul(
                        ps,
                        wk_sb[:, kk, :, ts(m, 128)],
                        xT8_tiles[b][:, kk, :, :],
                        start=(kk == 0),
                        stop=(kk == KK - 1),
                        perf_mode=DR,
                    )
                if ROWTILE:
                    kTp = kTp_tiles[b % 2]
                    nc.vector.tensor_copy(out=kTp[:, m, :], in_=ps)
                else:
                    kT2 = kT2_tiles[b % 2]
                    nc.vector.tensor_copy(out=kT2[0:64, 2 * m, :], in_=ps[0:64, :])
                    nc.vector.tensor_copy(
                        out=kT2[64:128, 2 * m + 1, :], in_=ps[64:128, :]
                    )

            def v_chain(b, i, half):
                """V chunk (i, half) with ones column at d=64."""
                if b not in v_tiles:
                    v_tiles[b] = xpool.tile(
                        [128, TCH, H, 65], bf16, name="v_sb", tag="v"
                    )
                    nc.vector.memset(v_tiles[b][:, :, :, 64:65], 1.0)
                v_sb = v_tiles[b]
                ps = psQ.tile([128, 512], f32, name="ps_v", tag="psq")
                for k in range(KO):
                    nc.tensor.matmul(
                        ps,
                        xT_tiles[b][:, k, ts(i, 128)],
                        wv_sb[:, k, ts(half, 512)],
                        start=(k == 0),
                        stop=(k == KO - 1),
                    )
                nc.vector.tensor_copy(
                    out=v_sb[:, i, 8 * half : 8 * half + 8, 0:64],
                    in_=ps.rearrange("p (h d) -> p h d", d=64),
                )

            def stage_a_chains(b):
                """All projection chains for batch b, in dependency-friendly
                order (Q first: scores need it first)."""
                chains = []
                for m in range(MO):
                    chains.append(lambda b=b, m=m: q_chain(b, m))
                for m in range(MO):
                    chains.append(lambda b=b, m=m: k_chain(b, m))
                for i in range(TCH):
                    for half in range(2):
                        chains.append(lambda b=b, i=i, half=half: v_chain(b, i, half))
                return chains

            def scores_pair(b, h):
                """Row-tiled scores^T -> exp -> masks for heads h, h+1
                (h even). Head h runs on PE rows 0:63, head h+1 on rows
                64:127, concurrently. Returns [(h, aT_h), (h+1, aT_h1)]."""
                qT_sb = qT_tiles[b]
                pair = h // 2
                if ROWTILE:
                    kTp = kTp_tiles[b % 2]
                    # j2s first: they carry the tightest WAR (on the AV
                    # normalize draining banks 4/5), so give them the whole
                    # pair-stream of slack before the next pair needs them
                    order = [(2, 0), (2, 1), (0, 0), (0, 1), (1, 0), (1, 1), (3, 0), (3, 1)]
                    for j, par in order:
                        w = widths[j]
                        lo, hi = 64 * par, 64 * par + 64
                        lhsT = kTp[lo:hi, pair, ts(j, 128)]
                        rhs = qT_sb[lo:hi, pair, ds(128 * j, w)]
                        if j == 2:
                            out_ap = psJ2[par][:, 0:w]
                        else:
                            out_ap = psS_par[par][:, ds(sOFF[j], w)]
                        nc.tensor.matmul(out_ap, lhsT, rhs, start=True, stop=True)
                else:
                    kT2 = kT2_tiles[b % 2]
                    for par in range(2):
                        hh = h + par
                        for j in range(TCH):
                            w = widths[j]
                            if j == 2:
                                out_ap = psJ2[par][:, 0:w]
                            else:
                                out_ap = psS_par[par][:, ds(sOFF[j], w)]
                            nc.tensor.matmul(
                                out_ap,
                                kT2[:, hh, ts(j, 128)],
                                qT_sb[:, pair, ds(128 * j, w)],
                                start=True,
                                stop=True,
                            )
                res = []
                for par in range(2):
                    hh = h + par
                    aT = apool.tile([128, PACK], bf16, name="aT", tag=f"aT{hh % 5}")
                    nc.scalar.activation(
                        aT[:, 0:1024], psS_par[par], AF.Exp, scale=SCALE
                    )
                    nc.scalar.activation(
                        aT[:, 1024:1280], psJ2[par][:, 0:256], AF.Exp, scale=SCALE
                    )
                    # zero the masked (s>t) part of the diagonal blocks
                    nc.gpsimd.tensor_mul(aT[:, 0:128], aT[:, 0:128], mask_sb)
                    nc.gpsimd.tensor_mul(aT[:, 512:1024], aT[:, 512:1024], m512_sb)
                    nc.gpsimd.tensor_mul(aT[:, 1024:1152], aT[:, 1024:1152], mask_sb)
                    res.append((hh, aT))
                return res

            def av_part(b, h, aT, on_sb):
                """attn @ [V | 1] for one head; all four t-chunks accumulate
                into the dedicated psAV bank (disjoint 68-wide ranges), so
                the head is normalized with one reciprocal + per-chunk
                scales -- emitted BEFORE any filler evac so the DVE drains
                psAV early."""
                v_sb = v_tiles[b]
                for i in range(TCH):
                    for j in range(i + 1):
                        nc.tensor.matmul(
                            psAV_chunk(i, 65),
                            aT[:, ds(off[j] + 128 * (i - j), 128)],
                            v_sb[:, j, h, :],
                            start=(j == 0),
                            stop=(j == i),
                        )
                rr = spool.tile([128, TCH, 1], f32, name="rr", tag="rr")
                # rowsums at col 64 of each chunk (banks 5 and 4)
                nc.vector.reciprocal(
                    rr[:, 0:3, :],
                    psB5[:, ds(256, 3 * 68)].rearrange("p (i c) -> p i c", c=68)[
                        :, :, 64:65
                    ],
                )
                nc.vector.reciprocal(rr[:, 3:4, :], psB4[:, ds(256 + 64, 1)])
                for i in range(TCH):
                    nc.vector.tensor_scalar_mul(
                        on_sb[:, i, ds(64 * h, 64)],
                        psAV_chunk(i, 64),
                        rr[:, i, :],
                    )

            def proj_chain(b, outT_sb, out_sb, i, half):
                """One output-projection chain; issues the row-chunk's output
                DMA after the second half."""
                psF = psQ.tile([128, 512], f32, name="psF", tag="psq")
                for k in range(MO):
                    nc.tensor.matmul(
                        psF,
                        outT_sb[:, k, ts(i, 128)],
                        wp_sb[:, k, ts(half, 512)],
                        start=(k == 0),
                        stop=(k == MO - 1),
                    )
                nc.vector.tensor_add(
                    out=out_sb[:, ts(half, 512)],
                    in0=psF,
                    in1=bias_bc[:, ts(half, 512)],
                )
                if half == 1:
                    nc.sync.dma_start(out=out[b, ts(i, 128), :], in_=out_sb)

            def epilogue_chains(b, outT_sb):
                chains = []
                state = {}
                for i in range(TCH):
                    for half in range(2):
                        def ch(b=b, outT_sb=outT_sb, i=i, half=half):
                            if i not in state:
                                state[i] = opool.tile(
                                    [128, C], bf16, name="out_sb", tag="out_sb"
                                )
                            proj_chain(b, outT_sb, state[i], i, half)
                        chains.append(ch)
                return chains

            # ---------- main software-pipelined loop ----------
            deferred_epi = None
            for b in range(b_loc):
                if b == 0:
                    # startup: run batch 0's projections at DMA pace (warm-up
                    # matmuls above cover the PE while data streams in)
                    for ch in stage_a_chains(0):
                        ch()
                last = b == b_loc - 1
                if last and deferred_epi is not None:
                    # the last batch has no next batch to interleave: its head
                    # slots are filled with the PREVIOUS batch's deferred
                    # output projection instead
                    next_chains = deferred_epi
                    head_quota = len(next_chains)
                    every = 2  # 8 chains spread over 19 slots
                else:
                    next_chains = stage_a_chains(b + 1) if b + 1 < b_loc else []
                    head_quota = 2 * MO
                    every = 1
                ci = 0

                on_sb = onpool.tile([128, TCH, HD], bf16, name="on_sb", tag="on")
                outT_sb = opool.tile([128, MO, T], bf16, name="outT_sb", tag="outT")
                # attention heads (AV lags scores by LAG slots), with filler
                # chains between. Slot order [AV(h-LAG), filler, scores(pair)]
                # matters: ACT PSUM-reads (exp) conservatively wait for ALL
                # matmuls scheduled before them, so scores must be the last
                # PE work ahead of its own exp in the stream.
                pend = []
                for h in range(H + LAG):
                    if h >= LAG:
                        ph, paT = pend.pop(0)
                        av_part(b, ph, paT, on_sb)
                        if ph == H // 2 - 1:
                            # heads 0-7 are done: transpose their half of the
                            # concat now (idle sync queue) so the projection
                            # only waits on the heads 8-15 half at the end
                            for i in range(TCH):
                                nc.sync.dma_start_transpose(
                                    out=outT_sb[:, 0 : MO // 2, ts(i, 128)],
                                    in_=on_sb[:, i, 0 : HD // 2],
                                )
                    if (
                        h >= LAG
                        and (h - LAG) % every == 0
                        and ci < head_quota
                        and ci < len(next_chains)
                    ):
                        next_chains[ci]()
                        ci += 1
                    if h < H and h % 2 == 0:
                        pend.extend(scores_pair(b, h))
                # second half of the head-concat transpose. Concurrent
                # transposes on two queues corrupt, so they ride the sync
                # queue -- except the LAST batch, whose second half goes to
                # the (by then exp-free) scalar queue: its sync first-half
                # finished long ago and the final projection gates on these.
                t_eng = nc.scalar if last else nc.sync
                for i in range(TCH):
                    t_eng.dma_start_transpose(
                        out=outT_sb[:, MO // 2 : MO, ts(i, 128)],
                        in_=on_sb[:, i, HD // 2 : HD],
                    )
                issue_xT(b + 2)
                if b == b_loc - 2 and b_loc >= 2:
                    # run the next batch's V chains now, and defer this
                    # batch's projection epilogue into the last batch's
                    # attention slots
                    while ci < len(next_chains):
                        next_chains[ci]()
                        ci += 1
                    deferred_epi = epilogue_chains(b, outT_sb)
                    continue
                # final projection (+ bias during PSUM evacuation), V filler
                for chain in epilogue_chains(b, outT_sb):
                    if ci < len(next_chains):
                        next_chains[ci]()
                        ci += 1
                    chain()
                # any leftover next-batch chains
                while ci < len(next_chains):
                    next_chains[ci]()
                    ci += 1

    nc.compile()
    return nc


def make_in_maps(x, wq, wk, wv, w_proj, b_proj, b_loc=B_LOC, ncores=NCORES):
    bf16 = ml_dtypes.bfloat16
    fp8 = ml_dtypes.float8_e4m3
    x = np.asarray(x, dtype=np.float32)
    # host-side layout prep (transpose / reshape / scale / cast only)
    xTf = np.ascontiguousarray(x.transpose(0, 2, 1))  # [B, C, T] f32
    xT = xTf.astype(bf16)
    xT8 = np.clip(xTf, -240, 240).astype(fp8)
    # wq/wk scaled by 32 into the e4m3 normal range (std 1/32 -> ~1);
    # the kernel divides the scores by 1024 inside the exp
    wq2 = np.clip(
        32.0 * np.asarray(wq, np.float32).transpose(1, 0, 2).reshape(C, HD),
        -240,
        240,
    ).astype(fp8)
    wk2 = np.clip(
        32.0 * np.asarray(wk, np.float32).transpose(1, 0, 2).reshape(C, HD),
        -240,
        240,
    ).astype(fp8)
    wv2 = np.ascontiguousarray(
        np.asarray(wv, np.float32).transpose(1, 0, 2).reshape(C, HD)
    ).astype(bf16)
    wp2 = np.ascontiguousarray(np.asarray(w_proj, np.float32)).astype(bf16)
    bp2 = np.asarray(b_proj, np.float32).reshape(1, C).astype(bf16)
    # mask[p, f] = 1 where p <= f (valid: s_in <= t_in on diagonal blocks)
    m = np.triu(np.ones((128, 128), np.float32))
    m512 = np.concatenate([m, np.ones((128, 256), np.float32), m], axis=1)
    in_maps = []
    for c in range(ncores):
        in_maps.append(
            {
                "xT": xT[c * b_loc : (c + 1) * b_loc],
                "xT8": xT8[c * b_loc : (c + 1) * b_loc],
                "wq": wq2,
                "wk": wk2,
                "wv": wv2,
                "wp": wp2,
                "bp": bp2,
                "mask": m.astype(bf16),
                "mask512": m512.astype(bf16),
            }
        )
    return in_maps


def kernel(x, wq, wk, wv, w_proj, b_proj, **run_kwargs):
    from concourse import bass_utils

    if "nc" not in _CACHE:
        _CACHE["nc"] = build_nc(B_LOC)
    nc = _CACHE["nc"]
    in_maps = make_in_maps(x, wq, wk, wv, w_proj, b_proj)
    res = bass_utils.run_bass_kernel_spmd(
        nc, in_maps, core_ids=list(range(NCORES)), **run_kwargs
    )
    outs = [r["out"] for r in res.results]
    full = np.concatenate(outs, axis=0).astype(np.float32)
    _CACHE["last_result"] = res
    return full


# revision 43
# speedup vs baseline: 1.0079x; 1.0079x over previous
"""Causal multi-head attention (B=32,T=512,C=1024,H=16,D=64) on 8 TRN2 cores.

Strategy: pure data-parallel over the batch axis (4 batches per core, no
collectives). Per core, per batch:
  - x^T [C,T] arrives pre-transposed from the host (layout prep only).
  - Q^T [HD,T] / K^T / V computed with bf16 matmuls (fp32 PSUM). K^T keeps
    its natural paired layout: head 2m on partitions 0:64, head 2m+1 on
    64:128 of kTp[:, m, :] -- a single [128,T] PSUM copy per chunk.
  - scores^T for a PAIR of heads computed with row-tiled matmuls: head 2m
    uses PE rows 0:63 (tile_position (0,0), K=64) and head 2m+1 rows 64:127
    (tile_position (64,0)) CONCURRENTLY -- 2x scores throughput vs the
    zero-padded K=128 form, and no kT zero-fill memsets.
  - scores PSUM layout: per-parity [128,1024] tile (j0@[0,512) bank A,
    j1@[512,896)+j3@[896,1024) bank B) plus a shared [128,2,256] tile for
    the j2 chunk (one bank, even/odd halves) -- exp is 2 ACTs per head into
    the same aT offsets as a packed [128,1280] tile would use.
  - softmax without max-subtraction (scores bounded); masked entries zeroed
    by multiplying exp'd diagonal blocks with 0/1 masks on GpSimd.
  - attn@V with a ones-augmented V column producing softmax row-sums in the
    same matmul; all four t-chunks accumulate into ONE dedicated PSUM bank
    (psAV, its own bank so projection-chain PSUM never WAR-waits on the
    normalize chain -- this was the baseline's 1.3us/slot PE stall).
  - head-concat transpose via batched DMA-transpose; final projection with
    bias folded into the PSUM evacuation; bf16 output (host casts to f32).

PSUM budget (8 banks): psS_even 2 + psS_odd 2 + psJ2 1 + psAV 1 + psq 2.

Scheduling:
  - initial DMAs spread across 4 queues (sync/scalar/vector/gpsimd) so the
    first Q matmul is gated by ~wq+xT arrival only.
  - warm-up junk matmuls at t=0 keep the PE busy during the initial DMA so
    the HAM clock-gate reaches 8/8 before real work lands.
  - software pipeline, two levels:
      * within attention: AV(h-3) issues while scores(pair) runs; scalar
        engine kept exp-only; DVE emission order per slot puts the AV
        normalize (recip + 4 scalar-muls) BEFORE filler-chain evacuations;
      * across batches: batch b+1's projection chains interleave batch b's
        attention head slots and output projection.
"""

import sys

if "/opt/trn_rl_repo" not in sys.path:
    sys.path.insert(0, "/opt/trn_rl_repo")

import numpy as np
import ml_dtypes

B, T, C = 32, 512, 1024
H, D = 16, 64
HD = H * D
NCORES = 8
B_LOC = B // NCORES

_CACHE = {}


def build_nc(b_loc=B_LOC):
    import concourse.mybir as mybir
    from concourse import bacc
    from concourse.bass import ds, ts
    from concourse.tile import TileContext

    f32 = mybir.dt.float32
    bf16 = mybir.dt.bfloat16
    fp8 = mybir.dt.float8e4
    DR = mybir.MatmulPerfMode.DoubleRow
    AF = mybir.ActivationFunctionType

    KO = C // 128  # 8 contraction chunks
    KK = C // 256  # 4 DoubleRow contraction chunks (256 rows each)
    MO = HD // 128  # 8 output-row chunks
    TCH = T // 128  # 4 t-chunks
    NPAIR = H // 2
    # wq/wk host-scaled by 32 => scores inflated 1024x; fold into exp
    SCALE = 1.0 / float(np.sqrt(C)) / 1024.0
    N_WARM = 26
    LAG = 3  # attention software-pipeline depth (AV trails scores by LAG)
    ROWTILE = False  # row-tiled K=64 pair scores vs zero-padded K=128

    # scores^T causal layout: s-chunk j covers t in [128j, T), width T-128j.
    # aT (SBUF) offsets: j0@0, j1@512, j3@896, j2@1024 (packed [128,1280]).
    # PSUM: j0/j1/j3 in a [128,1024] per-parity tile (2 banks, no matmul
    # crosses a bank), j2 (width 256) in a shared [128,2,256] one-bank tile.
    widths = [T - 128 * j for j in range(TCH)]
    off = [0, 512, 1024, 896]  # aT column offset per j
    sOFF = {0: 0, 1: 512, 3: 896}  # psS column offset per j (j2 -> psJ2)
    PACK = 1280

    nc = bacc.Bacc("TRN2", target_bir_lowering=False)
    xT = nc.dram_tensor("xT", [b_loc, C, T], bf16, kind="ExternalInput")
    # fp8 copies for the Q/K projections (DoubleRow, 2x PE throughput);
    # wq/wk arrive host-scaled by 32 so their N(0,1/1024) entries use the
    # e4m3 normal range -- the 1024x score inflation is folded into the
    # exp's scale. Softmax damping keeps the fp8 error ~1e-2 end to end.
    xT8 = nc.dram_tensor("xT8", [b_loc, C, T], fp8, kind="ExternalInput")
    wq = nc.dram_tensor("wq", [C, HD], fp8, kind="ExternalInput")
    wk = nc.dram_tensor("wk", [C, HD], fp8, kind="ExternalInput")
    wv = nc.dram_tensor("wv", [C, HD], bf16, kind="ExternalInput")
    wp = nc.dram_tensor("wp", [C, C], bf16, kind="ExternalInput")
    bp = nc.dram_tensor("bp", [1, C], bf16, kind="ExternalInput")
    mask = nc.dram_tensor("mask", [128, 128], bf16, kind="ExternalInput")
    # [tri | ones(256) | tri]: masks the j1 and j3 diagonal blocks (packed at
    # columns 512:640 and 896:1024 of aT) in one elementwise multiply.
    mask512 = nc.dram_tensor("mask512", [128, 512], bf16, kind="ExternalInput")
    out = nc.dram_tensor("out", [b_loc, T, C], bf16, kind="ExternalOutput")

    with TileContext(nc) as tc:
        with (
            tc.tile_pool(name="weights", bufs=1) as wpool,
            tc.tile_pool(name="acts", bufs=2) as xpool,
            # 3 slots for the x tiles: the b+2 prefetch DMA starts into a
            # free slot instead of WAR-stalling at the queue head on batch
            # b's readers (which also spuriously blocks later consumers via
            # lumped DMA-semaphore thresholds)
            tc.tile_pool(name="xacts", bufs=3) as x3pool,
            tc.tile_pool(name="attn", bufs=1) as apool,
            tc.tile_pool(name="small", bufs=8) as spool,
            tc.tile_pool(name="ons", bufs=2) as onpool,
            tc.tile_pool(name="outs", bufs=2) as opool,
            tc.tile_pool(name="psScores", bufs=1, space="PSUM") as psS,
            tc.tile_pool(name="psChain", bufs=2, space="PSUM") as psQ,
        ):
            # ---- persistent scores/AV PSUM tiles (bank-exact) ----
            # Row-tiled pairs: the two concurrent PE row-tiles must NEVER
            # touch the same PSUM bank (HW restriction), so even-head scores
            # go to banks 0,1,4 and odd-head scores to banks 2,3,5. The AV
            # accumulator (written by non-tiled matmuls, which never overlap
            # tiled ones thanks to the mode-switch drain) fills the back
            # halves of banks 4 and 5.
            psS_par = [
                psS.tile([128, 1024], f32, name="psS_e"),  # banks 0-1
                psS.tile([128, 1024], f32, name="psS_o"),  # banks 2-3
            ]
            # The two concurrent row tiles must have fully disjoint PSUM
            # bank sets (sharing one bank hangs the PE, even when the two
            # tiles' matmuls are emitted far apart). Non-tiled matmuls (AV)
            # may share a bank with a row tile (mode switches drain), so the
            # AV accumulator chunks fill the back halves of banks 4 and 5.
            psB4 = psS.tile([128, 512], f32, name="psB4")  # bank 4 (even j2)
            psB5 = psS.tile([128, 512], f32, name="psB5")  # bank 5 (odd j2)
            psJ2 = [psB4[:, 0:256], psB5[:, 0:256]]
            # AV chunks (68-wide, 16B-aligned): i=0..2 in bank 5, i=3 bank 4
            def psAV_chunk(i, w=65):
                if i < 3:
                    return psB5[:, ds(256 + 68 * i, w)]
                return psB4[:, ds(256, w)]

            # ---- persistent weight tiles ----
            # wq/wk in fp8 DoubleRow layout: [p, kk, i, n] with contraction
            # row c = kk*256 + i*128 + p
            wq_sb = wpool.tile([128, KK, 2, HD], fp8, name="wq_sb")
            wk_sb = wpool.tile([128, KK, 2, HD], fp8, name="wk_sb")
            wv_sb = wpool.tile([128, KO, HD], bf16, name="wv_sb")
            wp_sb = wpool.tile([128, KO, C], bf16, name="wp_sb")
            bp1_sb = wpool.tile([1, C], bf16, name="bp1_sb")
            mask_sb = wpool.tile([128, 128], bf16, name="mask_sb")
            m512_sb = wpool.tile([128, 512], bf16, name="m512_sb")
            # warm-up junk operand: zeros, written by the (fast) vector memset
            warm_sb = wpool.tile([128, 512], bf16, name="warm_sb")
            nc.vector.memset(warm_sb, 0.0)
            ones1_sb = wpool.tile([1, 128], bf16, name="ones1_sb")
            nc.gpsimd.memset(ones1_sb, 1.0)
            # K^T paired layout (head 2m -> partitions 0:64, 2m+1 -> 64:128);
            # two persistent slots for cross-batch overlap. No zero padding.
            if ROWTILE:
                kTp_tiles = [
                    wpool.tile([128, NPAIR, T], bf16, name=f"kTp_{s}")
                    for s in range(2)
                ]
            else:
                # zero-padded per-head K^T (K=128 matmul fallback)
                kT2_tiles = []
                for slot in range(2):
                    t_ = wpool.tile([128, H, T], bf16, name=f"kT2_{slot}")
                    nc.gpsimd.memset(t_, 0.0)
                    kT2_tiles.append(t_)

            # ---- initial DMA issues: weights on the sync queue, x on the
            # scalar queue -- two DMA channels in parallel. Steady-state xT
            # prefetches also ride scalar, keeping the sync queue free for
            # the concat-transposes that gate the output projection.
            nc.sync.dma_start(out=bp1_sb, in_=bp[:])
            nc.sync.dma_start(out=mask_sb, in_=mask[:])
            nc.sync.dma_start(out=m512_sb, in_=mask512[:])
            xT_tiles = {}
            xT8_tiles = {}
            nc.sync.dma_start(
                out=wq_sb[:, 0:2, :, :],
                in_=wq[0:512, :].rearrange("(kk i p) n -> p kk i n", p=128, i=2),
            )
            nc.sync.dma_start(
                out=wq_sb[:, 2:4, :, :],
                in_=wq[512:1024, :].rearrange("(kk i p) n -> p kk i n", p=128, i=2),
            )
            nc.sync.dma_start(
                out=wk_sb, in_=wk[:].rearrange("(kk i p) n -> p kk i n", p=128, i=2)
            )
            nc.sync.dma_start(
                out=wp_sb, in_=wp[:].rearrange("(ko p) n -> p ko n", p=128)
            )

            def issue_wv():
                # on the scalar queue, behind xT[0..1] (needed ~45us in)
                nc.scalar.dma_start(
                    out=wv_sb[:, 0:4, :],
                    in_=wv[0:512, :].rearrange("(ko p) n -> p ko n", p=128),
                )
                nc.scalar.dma_start(
                    out=wv_sb[:, 4:8, :],
                    in_=wv[512:1024, :].rearrange("(ko p) n -> p ko n", p=128),
                )

            def issue_xT(b):
                if b >= b_loc or b in xT_tiles:
                    return
                xT_tiles[b] = x3pool.tile([128, KO, T], bf16, name="xT_sb", tag="xT")
                xT8_tiles[b] = x3pool.tile(
                    [128, KK, 2, T], fp8, name="xT8_sb", tag="xT8"
                )
                nc.scalar.dma_start(
                    out=xT8_tiles[b],
                    in_=xT8[b].rearrange("(kk i p) t -> p kk i t", p=128, i=2),
                )
                nc.scalar.dma_start(
                    out=xT_tiles[b],
                    in_=xT[b].rearrange("(ko p) t -> p ko t", p=128),
                )

            issue_xT(0)
            issue_xT(1)
            issue_wv()

            # ---- PE warm-up: junk matmuls so the HAM clock-gate opens while
            # the initial DMA streams in. Output is never read.
            for w in range(N_WARM):
                psw = psQ.tile([128, 512], f32, name="psw", tag="psq")
                nc.tensor.matmul(
                    psw, warm_sb[:, 0:128], warm_sb, start=True, stop=True
                )

            # ---- bias broadcast to all 128 partitions, f32, built once ----
            bias_bc = wpool.tile([128, C], f32, name="bias_bc")
            for half in range(2):
                psb = psQ.tile([128, 512], f32, name="psb", tag="psq")
                nc.tensor.matmul(
                    psb, ones1_sb, bp1_sb[:, ts(half, 512)], start=True, stop=True
                )
                nc.vector.tensor_copy(out=bias_bc[:, ts(half, 512)], in_=psb)

            # ---------- pipeline-stage chain generators ----------
            qT_tiles = {}
            v_tiles = {}

            def q_chain(b, m):
                """Q^T projection chunk m -> qT[b][:, m, :] (DVE evac).
                fp8 DoubleRow: 4 matmuls of K=256 each."""
                if b not in qT_tiles:
                    qT_tiles[b] = xpool.tile(
                        [128, MO, T], bf16, name="qT_sb", tag="qT"
                    )
                ps = psQ.tile([128, T], f32, name="ps_q", tag="psq")
                for kk in range(KK):
                    nc.tensor.matmul(
                        ps,
                        wq_sb[:, kk, :, ts(m, 128)],
                        xT8_tiles[b][:, kk, :, :],
                        start=(kk == 0),
                        stop=(kk == KK - 1),
                        perf_mode=DR,
                    )
                nc.vector.tensor_copy(out=qT_tiles[b][:, m, :], in_=ps)

            def k_chain(b, m):
                """K^T chunk m -> kTp (paired layout IS the PSUM layout)."""
                ps = psQ.tile([128, T], f32, name="ps_k", tag="psq")
                for kk in range(KK):
                    nc.tensor.matmul(
                        ps,
                        wk_sb[:, kk, :, ts(m, 128)],
                        xT8_tiles[b][:, kk, :, :],
                        start=(kk == 0),
                        stop=(kk == KK - 1),
                        perf_mode=DR,
                    )
                if ROWTILE:
                    kTp = kTp_tiles[b % 2]
                    nc.vector.tensor_copy(out=kTp[:, m, :], in_=ps)
                else:
                    kT2 = kT2_tiles[b % 2]
                    nc.vector.tensor_copy(out=kT2[0:64, 2 * m, :], in_=ps[0:64, :])
                    nc.vector.tensor_copy(
                        out=kT2[64:128, 2 * m + 1, :], in_=ps[64:128, :]
                    )

            def v_chain(b, i, half):
                """V chunk (i, half) with ones column at d=64."""
                if b not in v_tiles:
                    v_tiles[b] = xpool.tile(
                        [128, TCH, H, 65], bf16, name="v_sb", tag="v"
                    )
                    nc.vector.memset(v_tiles[b][:, :, :, 64:65], 1.0)
                v_sb = v_tiles[b]
                ps = psQ.tile([128, 512], f32, name="ps_v", tag="psq")
                for k in range(KO):
                    nc.tensor.matmul(
                        ps,
                        xT_tiles[b][:, k, ts(i, 128)],
                        wv_sb[:, k, ts(half, 512)],
                        start=(k == 0),
                        stop=(k == KO - 1),
                    )
                nc.vector.tensor_copy(
                    out=v_sb[:, i, 8 * half : 8 * half + 8, 0:64],
                    in_=ps.rearrange("p (h d) -> p h d", d=64),
                )

            def stage_a_chains(b):
                """All projection chains for batch b, in dependency-friendly
                order (Q first: scores need it first)."""
                chains = []
                for m in range(MO):
                    chains.append(lambda b=b, m=m: q_chain(b, m))
                for m in range(MO):
                    chains.append(lambda b=b, m=m: k_chain(b, m))
                for i in range(TCH):
                    for half in range(2):
                        chains.append(lambda b=b, i=i, half=half: v_chain(b, i, half))
                return chains

            def scores_pair(b, h):
                """Row-tiled scores^T -> exp -> masks for heads h, h+1
                (h even). Head h runs on PE rows 0:63, head h+1 on rows
                64:127, concurrently. Returns [(h, aT_h), (h+1, aT_h1)]."""
                qT_sb = qT_tiles[b]
                pair = h // 2
                if ROWTILE:
                    kTp = kTp_tiles[b % 2]
                    # j2s first: they carry the tightest WAR (on the AV
                    # normalize draining banks 4/5), so give them the whole
                    # pair-stream of slack before the next pair needs them
                    order = [(2, 0), (2, 1), (0, 0), (0, 1), (1, 0), (1, 1), (3, 0), (3, 1)]
                    for j, par in order:
                        w = widths[j]
                        lo, hi = 64 * par, 64 * par + 64
                        lhsT = kTp[lo:hi, pair, ts(j, 128)]
                        rhs = qT_sb[lo:hi, pair, ds(128 * j, w)]
                        if j == 2:
                            out_ap = psJ2[par][:, 0:w]
                        else:
                            out_ap = psS_par[par][:, ds(sOFF[j], w)]
                        nc.tensor.matmul(out_ap, lhsT, rhs, start=True, stop=True)
                else:
                    kT2 = kT2_tiles[b % 2]
                    for par in range(2):
                        hh = h + par
                        for j in range(TCH):
                            w = widths[j]
                            if j == 2:
                                out_ap = psJ2[par][:, 0:w]
                            else:
                                out_ap = psS_par[par][:, ds(sOFF[j], w)]
                            nc.tensor.matmul(
                                out_ap,
                                kT2[:, hh, ts(j, 128)],
                                qT_sb[:, pair, ds(128 * j, w)],
                                start=True,
                                stop=True,
                            )
                res = []
                for par in range(2):
                    hh = h + par
                    aT = apool.tile([128, PACK], bf16, name="aT", tag=f"aT{hh % 5}")
                    nc.scalar.activation(
                        aT[:, 0:1024], psS_par[par], AF.Exp, scale=SCALE
                    )
                    nc.scalar.activation(
                        aT[:, 1024:1280], psJ2[par][:, 0:256], AF.Exp, scale=SCALE
                    )
                    # zero the masked (s>t) part of the diagonal blocks
                    nc.gpsimd.tensor_mul(aT[:, 0:128], aT[:, 0:128], mask_sb)
                    nc.gpsimd.tensor_mul(aT[:, 512:1024], aT[:, 512:1024], m512_sb)
                    nc.gpsimd.tensor_mul(aT[:, 1024:1152], aT[:, 1024:1152], mask_sb)
                    res.append((hh, aT))
                return res

            def av_part(b, h, aT, on_sb):
                """attn @ [V | 1] for one head; all four t-chunks accumulate
                into the dedicated psAV bank (disjoint 68-wide ranges), so
                the head is normalized with one reciprocal + per-chunk
                scales -- emitted BEFORE any filler evac so the DVE drains
                psAV early."""
                v_sb = v_tiles[b]
                for i in range(TCH):
                    for j in range(i + 1):
                        nc.tensor.matmul(
                            psAV_chunk(i, 65),
                            aT[:, ds(off[j] + 128 * (i - j), 128)],
                            v_sb[:, j, h, :],
                            start=(j == 0),
                            stop=(j == i),
                        )
                rr = spool.tile([128, TCH, 1], f32, name="rr", tag="rr")
                # rowsums at col 64 of each chunk (banks 5 and 4)
                nc.vector.reciprocal(
                    rr[:, 0:3, :],
                    psB5[:, ds(256, 3 * 68)].rearrange("p (i c) -> p i c", c=68)[
                        :, :, 64:65
                    ],
                )
                nc.vector.reciprocal(rr[:, 3:4, :], psB4[:, ds(256 + 64, 1)])
                for i in range(TCH):
                    nc.vector.tensor_scalar_mul(
                        on_sb[:, i, ds(64 * h, 64)],
                        psAV_chunk(i, 64),
                        rr[:, i, :],
                    )

            def proj_chain(b, outT_sb, out_sb, i, half):
                """One output-projection chain; issues the row-chunk's output
                DMA after the second half."""
                psF = psQ.tile([128, 512], f32, name="psF", tag="psq")
                for k in range(MO):
                    nc.tensor.matmul(
                        psF,
                        outT_sb[:, k, ts(i, 128)],
                        wp_sb[:, k, ts(half, 512)],
                        start=(k == 0),
                        stop=(k == MO - 1),
                    )
                nc.vector.tensor_add(
                    out=out_sb[:, ts(half, 512)],
                    in0=psF,
                    in1=bias_bc[:, ts(half, 512)],
                )
                if half == 1:
                    nc.sync.dma_start(out=out[b, ts(i, 128), :], in_=out_sb)

            def epilogue_chains(b, outT_sb):
                chains = []
                state = {}
                for i in range(TCH):
                    for half in range(2):
                        def ch(b=b, outT_sb=outT_sb, i=i, half=half):
                            if i not in state:
                                state[i] = opool.tile(
                                    [128, C], bf16, name="out_sb", tag="out_sb"
                                )
                            proj_chain(b, outT_sb, state[i], i, half)
                        chains.append(ch)
                return chains

            # ---------- main software-pipelined loop ----------
            deferred_epi = None
            for b in range(b_loc):
                if b == 0:
                    # startup: run batch 0's projections at DMA pace (warm-up
                    # matmuls above cover the PE while data streams in)
                    for ch in stage_a_chains(0):
                        ch()
                last = b == b_loc - 1
                if last and deferred_epi is not None:
                    # the last batch has no next batch to interleave: its head
                    # slots are filled with the PREVIOUS batch's deferred
                    # output projection instead
                    next_chains = deferred_epi
                    head_quota = len(next_chains)
                    every = 2  # 8 chains spread over 19 slots
                else:
                    next_chains = stage_a_chains(b + 1) if b + 1 < b_loc else []
                    head_quota = 2 * MO
                    every = 1
                ci = 0

                on_sb = onpool.tile([128, TCH, HD], bf16, name="on_sb", tag="on")
                outT_sb = opool.tile([128, MO, T], bf16, name="outT_sb", tag="outT")
                # attention heads (AV lags scores by LAG slots), with filler
                # chains between. Slot order [AV(h-LAG), filler, scores(pair)]
                # matters: ACT PSUM-reads (exp) conservatively wait for ALL
                # matmuls scheduled before them, so scores must be the last
                # PE work ahead of its own exp in the stream.
                pend = []
                for h in range(H + LAG):
                    if h >= LAG:
                        ph, paT = pend.pop(0)
                        av_part(b, ph, paT, on_sb)
                        if ph == H // 2 - 1:
                            # heads 0-7 are done: transpose their half of the
                            # concat now (idle sync queue) so the projection
                            # only waits on the heads 8-15 half at the end
                            for i in range(TCH):
                                nc.sync.dma_start_transpose(
                                    out=outT_sb[:, 0 : MO // 2, ts(i, 128)],
                                    in_=on_sb[:, i, 0 : HD // 2],
                                )
                    if (
                        h >= LAG
                        and (h - LAG) % every == 0
                        and ci < head_quota
                        and ci < len(next_chains)
                    ):
                        next_chains[ci]()
                        ci += 1
                    if h < H and h % 2 == 0:
                        pend.extend(scores_pair(b, h))
                # second half of the head-concat transpose. Concurrent
                # transposes on two queues corrupt, so they ride the sync
                # queue -- except the LAST batch, whose second half goes to
                # the (by then exp-free) scalar queue: its sync first-half
                # finished long ago and the final projection gates on these.
                t_eng = nc.scalar if last else nc.sync
                for i in range(TCH):
                    t_eng.dma_start_transpose(
                        out=outT_sb[:, MO // 2 : MO, ts(i, 128)],
                        in_=on_sb[:, i, HD // 2 : HD],
                    )
                issue_xT(b + 2)
                if b == b_loc - 2 and b_loc >= 2:
                    # run the next batch's V chains now, and defer this
                    # batch's projection epilogue into the last batch's
                    # attention slots
                    while ci < len(next_chains):
                        next_chains[ci]()
                        ci += 1
                    deferred_epi = epilogue_chains(b, outT_sb)
                    continue
                # final projection (+ bias during PSUM evacuation), V filler
                for chain in epilogue_chains(b, outT_sb):
                    if ci < len(next_chains):
                        next_chains[ci]()
                        ci += 1
                    chain()
                # any leftover next-batch chains
                while ci < len(next_chains):
                    next_chains[ci]()
                    ci += 1

    nc.compile()
    return nc


def make_in_maps(x, wq, wk, wv, w_proj, b_proj, b_loc=B_LOC, ncores=NCORES):
    bf16 = ml_dtypes.bfloat16
    fp8 = ml_dtypes.float8_e4m3
    x = np.asarray(x, dtype=np.float32)
    # host-side layout prep (transpose / reshape / scale / cast only)
    xTf = np.ascontiguousarray(x.transpose(0, 2, 1))  # [B, C, T] f32
    xT = xTf.astype(bf16)
    xT8 = np.clip(xTf, -240, 240).astype(fp8)
    # wq/wk scaled by 32 into the e4m3 normal range (std 1/32 -> ~1);
    # the kernel divides the scores by 1024 inside the exp
    wq2 = np.clip(
        32.0 * np.asarray(wq, np.float32).transpose(1, 0, 2).reshape(C, HD),
        -240,
        240,
    ).astype(fp8)
    wk2 = np.clip(
        32.0 * np.asarray(wk, np.float32).transpose(1, 0, 2).reshape(C, HD),
        -240,
        240,
    ).astype(fp8)
    wv2 = np.ascontiguousarray(
        np.asarray(wv, np.float32).transpose(1, 0, 2).reshape(C, HD)
    ).astype(bf16)
    wp2 = np.ascontiguousarray(np.asarray(w_proj, np.float32)).astype(bf16)
    bp2 = np.asarray(b_proj, np.float32).reshape(1, C).astype(bf16)
    # mask[p, f] = 1 where p <= f (valid: s_in <= t_in on diagonal blocks)
    m = np.triu(np.ones((128, 128), np.float32))
    m512 = np.concatenate([m, np.ones((128, 256), np.float32), m], axis=1)
    in_maps = []
    for c in range(ncores):
        in_maps.append(
            {
                "xT": xT[c * b_loc : (c + 1) * b_loc],
                "xT8": xT8[c * b_loc : (c + 1) * b_loc],
                "wq": wq2,
                "wk": wk2,
                "wv": wv2,
                "wp": wp2,
                "bp": bp2,
                "mask": m.astype(bf16),
                "mask512": m512.astype(bf16),
            }
        )
    return in_maps


def kernel(x, wq, wk, wv, w_proj, b_proj, **run_kwargs):
    from concourse import bass_utils

    if "nc" not in _CACHE:
        _CACHE["nc"] = build_nc(B_LOC)
    nc = _CACHE["nc"]
    in_maps = make_in_maps(x, wq, wk, wv, w_proj, b_proj)
    res = bass_utils.run_bass_kernel_spmd(
        nc, in_maps, core_ids=list(range(NCORES)), **run_kwargs
    )
    outs = [r["out"] for r in res.results]
    full = np.concatenate(outs, axis=0).astype(np.float32)
    _CACHE["last_result"] = res
    return full
